# revision 8
# baseline (speedup 1.0000x reference)
"""AttVlad Trainium2 kernel.

Math (per image n):
  xn = x / ||x||_2(over d)                       x: [D=128, S]
  a  = softmax_k(conv_w @ xn + conv_b)           a: [K=64, S]
  vlad[k,d]   = sum_s a[k,s] xn[d,s] - (sum_s a[k,s]) centroids[k,d]
  out = normalize_d(vlad * (centroids @ att_w.T + att_b))

Device strategy (8 cores, data-parallel over n, 4 images each):
  - x[n] is streamed in [128d, 2048s] chunks (fp32->bf16 cast during DMA).
  - Per 128-s unit: one PE pass with lhsT = x_chunk slice produces BOTH
    logits^T [128s, 64k] (rhs = conv_w^T) and x^T [128s, 128d] (rhs = I).
  - All per-s scalars (rsqrt of sumsq, softmax denom, their products) live
    as [128, 16] tiles (s on partitions) and are applied via broadcast
    (step-0) access patterns, so softmax needs no per-unit scalar ops.
  - Normalization scalars never touch x: logits are scaled by rnorm before
    exp; the VLAD matmul uses lhsT a2 = exp(l*rnorm)*exp(b) * (rnorm*rdenom)
    and rhs = [x^T | norm], giving columns [A | asum] accumulated in PSUM.
  - rsqrt is computed as exp(-0.5*ln(s)) to stay inside one ACT table set.
  - Host does the O(N*K*D) finalize (centroid subtract, attention scale,
    intra-normalization) in float64.
"""

import sys

import numpy as np

try:  # the concourse stack (bass) ships in the container image
    import concourse.bass as _probe  # noqa: F401
except Exception:  # pragma: no cover
    sys.path.insert(0, "/opt/trn_rl_repo")

import ml_dtypes

N, D, S, K = 32, 128, 16384, 64
NCORES = 8
EPS = 1e-12

CHUNK = 2048  # s-positions per DMA chunk
UNIT = 128    # s-positions per matmul unit (psum partition dim)
XT_STRIDE = 130  # x^T unit stride in the SBUF tile: 128 cols x^T + 1 norm + 1 pad


def _make_tile_context_cls(tile, mybir, ScopedClock):
    """This walrus build rejects instructions carrying more than one sync
    wait; excess waits are split onto same-engine NoOps by _split_waits."""
    return tile.TileContext


MAX_WAITS = 1


def _split_waits(nc, mybir):
    """Rewrite the traced BIR so no instruction carries more than MAX_WAITS
    sem waits: excess waits move to injected NoOps immediately preceding the
    instruction on the same engine (NX executes waits in order, so this is
    semantically identical)."""
    nid = 0
    for f in nc.m.functions:
        for blk in f.blocks:
            new_insts = []
            for inst in blk.instructions:
                si = getattr(inst, "sync_info", None)
                ws = list(si.on_wait) if si is not None else []
                if len(ws) > MAX_WAITS:
                    for i in range(0, len(ws) - MAX_WAITS, MAX_WAITS):
                        nid += 1
                        nop = mybir.InstNoOp(
                            name=f"waitsplit_{nid}", ins=[], outs=[]
                        )
                        nop.engine = inst.engine
                        nop.sync_info = mybir.SyncInfo(
                            on_wait=ws[i : i + MAX_WAITS], on_update=[]
                        )
                        new_insts.append(nop)
                    si.on_wait = ws[len(ws) - MAX_WAITS :]
                new_insts.append(inst)
            blk.instructions[:] = new_insts


def build_program(n_per_core=4, s_total=S):
    """Build the single-core Bass program (same program runs on all cores)."""
    import concourse.bass as bass
    import concourse.tile as tile
    from concourse import mybir
    from concourse.vector_clock import ScopedClock

    dt = mybir.dt
    AF = mybir.ActivationFunctionType
    OP = mybir.AluOpType

    TileContextFixed = _make_tile_context_cls(tile, mybir, ScopedClock)

    n_chunks = s_total // CHUNK
    units_per_chunk = CHUNK // UNIT          # 16
    halves = (0, 1)                          # 8 units each
    HU = units_per_chunk // 2                # units per half

    nc = bass.Bass()
    x_in = nc.declare_dram_parameter(
        "x", [n_per_core, D, s_total], dt.float32, isOutput=False
    )
    wt_in = nc.declare_dram_parameter("wt", [D, K], dt.bfloat16, isOutput=False)
    idm_in = nc.declare_dram_parameter("idm", [D, D], dt.bfloat16, isOutput=False)
    expb_in = nc.declare_dram_parameter(
        "expb", [128, (units_per_chunk // 2) * K], dt.bfloat16, isOutput=False
    )
    out_dram = nc.declare_dram_parameter(
        "out", [K, n_per_core * 132], dt.float32, isOutput=True
    )

    with TileContextFixed(nc) as tc:
        with (
            tc.tile_pool(name="consts", bufs=1) as consts,
            tc.tile_pool(name="xc", bufs=3) as xc_pool,
            tc.tile_pool(name="xt", bufs=3) as xt_pool,
            tc.tile_pool(name="soft", bufs=3) as soft_pool,
            tc.tile_pool(name="stats", bufs=2) as stats_pool,
            tc.tile_pool(name="scratch", bufs=2) as scratch_pool,
            tc.tile_pool(name="outp", bufs=1) as out_pool,
            tc.tile_pool(name="psl", bufs=3, space="PSUM") as psl_pool,
            tc.tile_pool(name="pst", bufs=2, space="PSUM") as pst_pool,
            tc.tile_pool(name="pv", bufs=1, space="PSUM") as pv_pool,
        ):
            wt = consts.tile([D, K], dt.bfloat16)
            nc.sync.dma_start(wt[:], wt_in[:])
            idm = consts.tile([D, D], dt.bfloat16)
            nc.sync.dma_start(idm[:], idm_in[:])
            expb = consts.tile([128, HU * K], dt.bfloat16)
            nc.sync.dma_start(expb[:], expb_in[:])

            out_sb = out_pool.tile([K, n_per_core * 132], dt.float32)

            for n in range(n_per_core):
                pv = pv_pool.tile([K, 132], dt.float32)
                for ci in range(n_chunks):
                    xc = xc_pool.tile([D, CHUNK], dt.bfloat16)
                    # SWDGE cast-DMA: fp32 HBM -> bf16 SBUF
                    nc.gpsimd.dma_start(
                        xc[:], x_in[n, :, ci * CHUNK : (ci + 1) * CHUNK]
                    )

                    ss = stats_pool.tile([128, units_per_chunk], dt.float32, tag="ss")
                    rn = stats_pool.tile([128, units_per_chunk], dt.float32, tag="rn")
                    lns = stats_pool.tile([128, units_per_chunk], dt.float32, tag="lns")

                    psls = []
                    xts = []
                    for h in halves:
                        psl = psl_pool.tile([128, HU * K], dt.float32)
                        pst = pst_pool.tile([128, HU * D], dt.float32)
                        for u in range(HU):
                            cu = h * HU + u
                            lhsT = xc[:, cu * UNIT : (cu + 1) * UNIT]
                            nc.tensor.matmul(
                                psl[:, u * K : (u + 1) * K], lhsT, wt[:],
                                start=True, stop=True,
                            )
                            nc.tensor.matmul(
                                pst[:, u * D : (u + 1) * D], lhsT, idm[:],
                                start=True, stop=True,
                            )
                        # batched PSUM->SBUF move of x^T (bf16), strided per unit
                        xt = xt_pool.tile([128, HU * XT_STRIDE], dt.bfloat16)
                        xt3 = xt[:].rearrange("p (u c) -> p u c", c=XT_STRIDE)
                        pst3 = pst[:].rearrange("p (u c) -> p u c", c=D)
                        nc.scalar.activation(xt3[:, :, 0:D], pst3, AF.Copy)
                        # sum of squares over d -> ss columns (square, then
                        # free-dim reduce; both batched over the half)
                        sq = scratch_pool.tile([128, HU * D], dt.bfloat16, tag="sq")
                        sq3 = sq[:].rearrange("p (u c) -> p u c", c=D)
                        nc.vector.tensor_tensor(
                            out=sq3, in0=xt3[:, :, 0:D], in1=xt3[:, :, 0:D],
                            op=OP.mult,
                        )
                        nc.vector.tensor_reduce(
                            out=ss[:, h * HU : (h + 1) * HU],
                            in_=sq3, axis=mybir.AxisListType.X, op=OP.add,
                        )
                        psls.append(psl)
                        xts.append(xt)

                    # rnorm = exp(-0.5 * ln(sumsq))  (stays in one ACT table set)
                    nc.scalar.activation(lns[:], ss[:], AF.Ln)
                    nc.scalar.activation(rn[:], lns[:], AF.Exp, scale=-0.5)

                    for h in halves:
                        psl, xt = psls[h], xts[h]
                        xt3 = xt[:].rearrange("p (u c) -> p u c", c=XT_STRIDE)
                        rnh = rn[:, h * HU : (h + 1) * HU]
                        ssh = ss[:, h * HU : (h + 1) * HU]
                        # norm column (asum rhs): norm = sumsq * rnorm
                        nc.vector.tensor_tensor(
                            out=xt3[:, :, D : D + 1],
                            in0=ssh[:, :, None], in1=rnh[:, :, None],
                            op=OP.mult,
                        )
                        # l_scaled = logits_raw * rnorm (broadcast over k)
                        lsc = soft_pool.tile([128, HU * K], dt.bfloat16, tag="lsc")
                        nc.vector.tensor_tensor(
                            out=lsc[:].rearrange("p (u k) -> p u k", k=K),
                            in0=psl[:].rearrange("p (u k) -> p u k", k=K),
                            in1=rnh.broadcast_to([128, HU, K]),
                            op=OP.mult,
                        )
                        e = soft_pool.tile([128, HU * K], dt.bfloat16, tag="e")
                        nc.scalar.activation(e[:], lsc[:], AF.Exp)
                        # a_un = e * exp(b); denom = sum_k a_un (both batched)
                        a_un = soft_pool.tile([128, HU * K], dt.bfloat16, tag="a_un")
                        nc.vector.tensor_tensor(
                            out=a_un[:], in0=e[:], in1=expb[:], op=OP.mult
                        )
                        dn = stats_pool.tile([128, HU], dt.float32, tag="dn")
                        nc.vector.tensor_reduce(
                            out=dn[:],
                            in_=a_un[:].rearrange("p (u k) -> p u k", k=K),
                            axis=mybir.AxisListType.X, op=OP.add,
                        )
                        rdn = stats_pool.tile([128, HU], dt.float32, tag="rdn")
                        nc.vector.reciprocal(rdn[:], dn[:])
                        cc = stats_pool.tile([128, HU], dt.float32, tag="cc")
                        nc.vector.tensor_tensor(
                            out=cc[:], in0=rnh, in1=rdn[:], op=OP.mult
                        )
                        # a2 = a_un * (rnorm * rdenom), broadcast over k
                        a2 = soft_pool.tile([128, HU * K], dt.bfloat16, tag="a2")
                        nc.vector.tensor_tensor(
                            out=a2[:].rearrange("p (u k) -> p u k", k=K),
                            in0=a_un[:].rearrange("p (u k) -> p u k", k=K),
                            in1=cc[:].broadcast_to([128, HU, K]),
                            op=OP.mult,
                        )
                        # VLAD accumulation: pv[:, :129] += a2_u^T @ [x^T | norm]
                        for u in range(HU):
                            cu = ci * units_per_chunk + h * HU + u
                            first = cu == 0
                            last = cu == (s_total // UNIT) - 1
                            nc.tensor.matmul(
                                pv[:, 0 : D + 1],
                                a2[:, u * K : (u + 1) * K],
                                xt[:, u * XT_STRIDE : u * XT_STRIDE + D + 1],
                                start=first, stop=last,
                            )
                # stash [A | asum] for this n
                nc.scalar.activation(
                    out_sb[:, n * 132 : n * 132 + D + 1], pv[:, 0 : D + 1], AF.Copy
                )
            nc.sync.dma_start(out_dram[:], out_sb[:])

    _split_waits(nc, mybir)
    return nc


_CACHE = {}


def _get_program(n_per_core, s_total):
    key = (n_per_core, s_total)
    if key not in _CACHE:
        _CACHE[key] = build_program(n_per_core, s_total)
    return _CACHE[key]


def run_device(x, conv_w, conv_b, n_per_core=4, s_total=S, trace=False):
    """Run the device part. x: [NCORES*n_per_core, D, s_total] fp32.
    Returns (A [n, K, D], asum [n, K], bass_results)."""
    from concourse.bass_utils import run_bass_kernel_spmd

    nc = _get_program(n_per_core, s_total)

    bf16 = ml_dtypes.bfloat16
    wt_np = np.ascontiguousarray(conv_w.T.astype(bf16))           # [D, K]
    idm_np = np.eye(D, dtype=bf16)                                 # [D, D]
    expb_row = np.exp(conv_b.astype(np.float64)).astype(bf16)
    expb_np = np.broadcast_to(
        np.tile(expb_row, 8)[None, :], (128, 8 * K)
    ).copy()

    in_maps = []
    for c in range(NCORES):
        xc = np.ascontiguousarray(x[c * n_per_core : (c + 1) * n_per_core])
        in_maps.append({"x": xc, "wt": wt_np, "idm": idm_np, "expb": expb_np})

    res = run_bass_kernel_spmd(
        nc, in_maps, list(range(NCORES)), trace=trace,
    )

    n_total = NCORES * n_per_core
    A = np.empty((n_total, K, D), np.float64)
    asum = np.empty((n_total, K), np.float64)
    for c in range(NCORES):
        o = res.results[c]["out"]  # [K, n_per_core*132]
        for nl in range(n_per_core):
            blk = o[:, nl * 132 : nl * 132 + D + 1].astype(np.float64)
            A[c * n_per_core + nl] = blk[:, :D]
            asum[c * n_per_core + nl] = blk[:, D]
    return A, asum, res


def finalize(A, asum, centroids, att_w, att_b):
    cen = centroids.astype(np.float64)
    vlad = A - asum[:, :, None] * cen[None]
    soft = cen @ att_w.astype(np.float64).T + att_b.astype(np.float64)  # [K, 1]
    av = vlad * soft[None]
    nrm = np.maximum(np.linalg.norm(av, axis=2, keepdims=True), EPS)
    return (av / nrm).astype(np.float32)


def kernel(x, conv_w, conv_b, centroids, att_w, att_b):
    x = np.asarray(x, np.float32)
    A, asum, _ = run_device(
        x, np.asarray(conv_w, np.float32), np.asarray(conv_b, np.float32)
    )
    return finalize(
        A, asum,
        np.asarray(centroids, np.float32),
        np.asarray(att_w, np.float32),
        np.asarray(att_b, np.float32),
    )
